# revision 12
# baseline (speedup 1.0000x reference)
"""Fused pre-norm multi-head attention block on 8 TRN2 NeuronCores.

Sharding: data-parallel over (batch, sequence-half): core c owns batch c//2,
query rows (c%2)*1024..+1024. Zero inter-core communication: each core
computes K/V for its batch's (mask-compacted) key set locally.

Key compaction: masked keys get exactly 0 attention weight in the reference
(-inf scores), so we gather only unmasked key rows on the host (numpy), pad
to a multiple of 128 with bias -30 (exp(-30+s) ~ 1e-11, negligible), and run
dense attention over J ~= 1152 instead of 2048 keys.

Device pipeline (per core), all matmuls bf16 with f32 PSUM accumulation:
  A: LN(x) rows (f32 stats, Ln/Exp rsqrt chain) -> xn bf16 -> DMA-transpose
  B: Q/K/V projections (xn^T stationary, W^T moving) -> QK-layernorm per head
     (fold DH^-0.5 into Q's rsqrt) -> DMA-transpose Q_ln/K_ln; V stays natural
     with a ones-column per head (denominator trick)
  C: per head: S^T = K_ln^T.T @ Q_ln^T (2 heads packed in the 128-row PE via
     tile_position), exp with per-partition mask bias on ScalarE, O^T + denom
     accumulated via V_aug-stationary matmuls; normalize on PSUM eviction
  D: out = O_norm^T.T @ Wo^T -> f32 out
"""

import numpy as np
import ml_dtypes

import concourse.bacc as bacc
import concourse.bass as bass
import concourse.mybir as mybir
from concourse.tile import TileContext
from concourse.bass_utils import run_bass_kernel_spmd

BF16 = ml_dtypes.bfloat16
F32 = mybir.dt.float32
BF = mybir.dt.bfloat16
AF = mybir.ActivationFunctionType
ALU = mybir.AluOpType
AX = mybir.AxisListType

B, N, D, H, DH = 4, 2048, 1024, 16, 64
NQ = N // 2          # query rows per core
EPS = 1e-5
MASK_BIAS = -30.0


def build_kernel(J: int, reps: int = 1):
    """Build the per-core Bass graph. J = padded key count (multiple of 128)."""
    assert J % 128 == 0
    Jt = J // 128
    nc = bacc.Bacc()

    xq_d = nc.declare_dram_parameter("xq", [NQ, D], F32, isOutput=False)
    xkv_d = nc.declare_dram_parameter("xkv", [J, D], F32, isOutput=False)
    wqT_d = nc.declare_dram_parameter("wqT", [D, D], BF, isOutput=False)
    wkT_d = nc.declare_dram_parameter("wkT", [D, D], BF, isOutput=False)
    wvT_d = nc.declare_dram_parameter("wvT", [D, D], BF, isOutput=False)
    woT_d = nc.declare_dram_parameter("woT", [D, D], BF, isOutput=False)
    biasT_d = nc.declare_dram_parameter("biasT", [128, Jt], F32, isOutput=False)
    out_d = nc.declare_dram_parameter("out", [NQ, D], F32, isOutput=True)

    # DRAM scratch for partition-broadcast of softmax reciprocal denominators
    scr = [nc.dram_tensor(f"scr{h}", [1, NQ], F32) for h in range(H)]

    from contextlib import ExitStack

    with TileContext(nc) as tc:
        loop_ctx = tc.For_i(0, reps) if reps > 1 else None
        if loop_ctx is not None:
            loop_ctx.__enter__()
        try:
            with ExitStack() as ctx:
                _body(ctx, tc, nc, J, Jt,
                      xq_d, xkv_d, wqT_d, wkT_d, wvT_d, woT_d, biasT_d, out_d,
                      scr)
        finally:
            if loop_ctx is not None:
                loop_ctx.__exit__(None, None, None)
    nc.finalize()
    return nc


def _body(ctx, tc, nc, J, Jt, xq_d, xkv_d, wqT_d, wkT_d, wvT_d, woT_d,
          biasT_d, out_d, scr):
    from contextlib import ExitStack

    NQt = NQ // 128

    # ---- long-lived SBUF tensors ----
    statics = ctx.enter_context(tc.tile_pool(name="statics", bufs=1))
    xqT = statics.tile([128, 8, NQ], BF, tag="xqT")     # xn_q^T  [d, nq]
    xkvT = statics.tile([128, 8, J], BF, tag="xkvT")    # xn_kv^T [d, j]
    QT = statics.tile([128, 8, NQ], BF, tag="QT")       # Q_ln^T  [e, i]
    KT = statics.tile([128, 8, J], BF, tag="KT")        # K_ln^T  [e, j]
    Vaug = statics.tile([128, Jt, H, DH + 1], BF, tag="Vaug")  # V | ones
    OT = statics.tile([128, 8, NQ], BF, tag="OT")       # O_norm^T [f, i]
    bias_sb = statics.tile([128, Jt], F32, tag="bias")
    nc.sync.dma_start(out=bias_sb[:], in_=biasT_d[:])
    eps1 = statics.tile([128, 1], F32, tag="eps1")
    eps64 = statics.tile([128, 1], F32, tag="eps64")
    nc.vector.memset(eps1[:], EPS)
    nc.vector.memset(eps64[:], float(DH * EPS))

    for h in range(H):
        nc.vector.memset(Vaug[:, :, h, DH:DH + 1], 1.0)

    # ---- phase A: layernorm x rows + transpose ----
    with ExitStack() as actx:
        xpool = actx.enter_context(tc.tile_pool(name="xpool", bufs=3))
        xnpool = actx.enter_context(tc.tile_pool(name="xnpool", bufs=3))
        aspool = actx.enter_context(tc.tile_pool(name="aspool", bufs=4))

        def ln_rows(src_d, ntiles, dstT):
            for nt in range(ntiles):
                xt = xpool.tile([128, D], F32, tag="xt")
                nc.sync.dma_start(out=xt[:], in_=src_d[nt * 128:(nt + 1) * 128, :])
                st = aspool.tile([128, 2, 6], F32, tag="st")
                mv = aspool.tile([128, 2], F32, tag="mv")
                rr = aspool.tile([128, 1], F32, tag="rr")
                nc.vector.bn_stats(out=st[:, 0, :], in_=xt[:, 0:512])
                nc.vector.bn_stats(out=st[:, 1, :], in_=xt[:, 512:1024])
                nc.vector.bn_aggr(out=mv[:], in_=st[:])
                # r = (var+eps)^-0.5 via ACT (Ln then Exp; Rsqrt is banned)
                nc.scalar.activation(out=rr[:], in_=mv[:, 1:2], func=AF.Ln,
                                     bias=eps1[:], scale=1.0)
                nc.scalar.activation(out=rr[:], in_=rr[:], func=AF.Exp,
                                     bias=0.0, scale=-0.5)
                xn = xnpool.tile([128, D], BF, tag="xn")
                nc.vector.tensor_scalar(out=xn[:], in0=xt[:],
                                        scalar1=mv[:, 0:1], scalar2=rr[:],
                                        op0=ALU.subtract, op1=ALU.mult)
                nc.sync.dma_start_transpose(
                    out=dstT[:, :, nt * 128:(nt + 1) * 128], in_=xn[:])

        ln_rows(xkv_d, Jt, xkvT)
        ln_rows(xq_d, NQt, xqT)

    # ---- phase B: projections + QK layernorm ----
    bctx = ExitStack()
    wpool = bctx.enter_context(tc.tile_pool(name="wpool", bufs=1))
    wq_sb = wpool.tile([128, 8, D], BF, tag="wq")
    wk_sb = wpool.tile([128, 8, D], BF, tag="wk")
    wv_sb = wpool.tile([128, 8, D], BF, tag="wv")
    nc.sync.dma_start(out=wq_sb[:], in_=wqT_d[:].rearrange("(t p) e -> p t e", p=128))
    nc.sync.dma_start(out=wk_sb[:], in_=wkT_d[:].rearrange("(t p) e -> p t e", p=128))
    nc.sync.dma_start(out=wv_sb[:], in_=wvT_d[:].rearrange("(t p) e -> p t e", p=128))

    def qk_ln_chunk(pool, raw, qn, c, m_all, r_all, is_q):
        """QK layernorm of one [128, 512] chunk (8 head groups of 64).

        raw: bf16 SBUF [128, 512]; qn: bf16 out tile [128, 1024] (writes c-half).
        Computes r = rsqrt(var+eps) (times DH^-0.5 for Q, folded into the Ln
        scale/bias) and applies (x - m) * r per group.
        """
        g8 = raw[:].rearrange("p (g s) -> p g s", s=DH)
        sq = pool.tile([128, 512], BF, tag="sq")
        nc.vector.tensor_mul(out=sq[:], in0=raw[:], in1=raw[:])
        sums = pool.tile([128, 8], F32, tag="sums")
        sqs = pool.tile([128, 8], F32, tag="sqs")
        nc.vector.reduce_sum(out=sums[:], in_=g8, axis=AX.X)
        nc.vector.reduce_sum(out=sqs[:], in_=sq[:].rearrange("p (g s) -> p g s", s=DH), axis=AX.X)
        m = pool.tile([128, 8], F32, tag="m")
        nc.vector.tensor_scalar_mul(out=m[:], in0=sums[:], scalar1=1.0 / DH)
        msq64 = pool.tile([128, 8], F32, tag="msq64")
        nc.vector.tensor_mul(out=msq64[:], in0=m[:], in1=sums[:])  # 64*m^2
        t64 = pool.tile([128, 8], F32, tag="t64")
        nc.vector.tensor_sub(out=t64[:], in0=sqs[:], in1=msq64[:])
        r = pool.tile([128, 8], F32, tag="r")
        if is_q:
            # r = rsqrt(DH*(v+eps)) = rsqrt(v+eps)*DH^-0.5 ; DH*v = t64
            nc.scalar.activation(out=r[:], in_=t64[:], func=AF.Ln,
                                 bias=eps64[:], scale=1.0)
        else:
            nc.scalar.activation(out=r[:], in_=t64[:], func=AF.Ln,
                                 bias=eps1[:], scale=1.0 / DH)
        nc.scalar.activation(out=r[:], in_=r[:], func=AF.Exp, bias=0.0, scale=-0.5)
        for g in range(8):
            nc.vector.tensor_scalar(
                out=qn[:, c * 512 + g * 64: c * 512 + (g + 1) * 64],
                in0=g8[:, g, :], scalar1=m[:, g:g + 1], scalar2=r[:, g:g + 1],
                op0=ALU.subtract, op1=ALU.mult)

    # --- Q projection ---
    with ExitStack() as qctx:
        psQ = qctx.enter_context(tc.tile_pool(name="psQ", bufs=3, space="PSUM"))
        qpool = qctx.enter_context(tc.tile_pool(name="qpool", bufs=3))
        qstat = qctx.enter_context(tc.tile_pool(name="qstat", bufs=3))
        for nt in range(NQt):
            qp = psQ.tile([128, 1024], F32, tag="qp")
            for dt in range(8):
                lhs = xqT[:, dt, nt * 128:(nt + 1) * 128]
                nc.tensor.matmul(qp[:, 0:512], lhs, wq_sb[:, dt, 0:512],
                                 start=(dt == 0), stop=(dt == 7))
                nc.tensor.matmul(qp[:, 512:1024], lhs, wq_sb[:, dt, 512:1024],
                                 start=(dt == 0), stop=(dt == 7), skip_group_check=True)
            qn = qpool.tile([128, 1024], BF, tag="qn")
            for c in range(2):
                raw = qpool.tile([128, 512], BF, tag="qraw")
                nc.scalar.activation(out=raw[:], in_=qp[:, c * 512:(c + 1) * 512],
                                     func=AF.Copy)
                qk_ln_chunk(qstat, raw, qn, c, None, None, is_q=True)
            nc.sync.dma_start_transpose(
                out=QT[:, :, nt * 128:(nt + 1) * 128], in_=qn[:])

    # --- K and V projections ---
    with ExitStack() as kctx:
        psK = kctx.enter_context(tc.tile_pool(name="psK", bufs=2, space="PSUM"))
        psV = kctx.enter_context(tc.tile_pool(name="psV", bufs=2, space="PSUM"))
        kpool = kctx.enter_context(tc.tile_pool(name="kpool", bufs=3))
        kstat = kctx.enter_context(tc.tile_pool(name="kstat", bufs=3))
        for nt in range(Jt):
            kp = psK.tile([128, 1024], F32, tag="kp")
            vp = psV.tile([128, 1024], F32, tag="vp")
            for dt in range(8):
                lhs = xkvT[:, dt, nt * 128:(nt + 1) * 128]
                nc.tensor.matmul(kp[:, 0:512], lhs, wk_sb[:, dt, 0:512],
                                 start=(dt == 0), stop=(dt == 7))
                nc.tensor.matmul(kp[:, 512:1024], lhs, wk_sb[:, dt, 512:1024],
                                 start=(dt == 0), stop=(dt == 7), skip_group_check=True)
                nc.tensor.matmul(vp[:, 0:512], lhs, wv_sb[:, dt, 0:512],
                                 start=(dt == 0), stop=(dt == 7), skip_group_check=True)
                nc.tensor.matmul(vp[:, 512:1024], lhs, wv_sb[:, dt, 512:1024],
                                 start=(dt == 0), stop=(dt == 7), skip_group_check=True)
            kn = kpool.tile([128, 1024], BF, tag="kn")
            for c in range(2):
                raw = kpool.tile([128, 512], BF, tag="kraw")
                nc.scalar.activation(out=raw[:], in_=kp[:, c * 512:(c + 1) * 512],
                                     func=AF.Copy)
                qk_ln_chunk(kstat, raw, kn, c, None, None, is_q=False)
            nc.sync.dma_start_transpose(
                out=KT[:, :, nt * 128:(nt + 1) * 128], in_=kn[:])
            # V: evict into augmented per-head slots (leave ones column intact)
            nc.vector.tensor_copy(
                out=Vaug[:, nt, :, 0:DH],
                in_=vp[:].rearrange("p (h s) -> p h s", s=DH))

    bctx.close()  # free Wq/Wk/Wv SBUF before attention

    # ---- phase C: attention per head pair ----
    wopool = ctx.enter_context(tc.tile_pool(name="wopool", bufs=1))
    wo_sb = wopool.tile([128, 8, D], BF, tag="wo")
    nc.sync.dma_start(out=wo_sb[:], in_=woT_d[:].rearrange("(t p) e -> p t e", p=128))
    cctx = ExitStack()
    cpool = cctx.enter_context(tc.tile_pool(name="cpool", bufs=4))
    dpool = cctx.enter_context(tc.tile_pool(name="dpool", bufs=2))
    psS = cctx.enter_context(tc.tile_pool(name="psS", bufs=1, space="PSUM"))
    psO = cctx.enter_context(tc.tile_pool(name="psO", bufs=1, space="PSUM"))

    for p in range(8):
        ha, hb = 2 * p, 2 * p + 1
        Oa = psO.tile([65, 1024], F32, tag="Oa")
        Ob = psO.tile([65, 1024], F32, tag="Ob")
        for jt in range(Jt):
            Sa = psS.tile([128, 1024], F32, tag="Sa")
            Sb = psS.tile([128, 1024], F32, tag="Sb")
            kta = KT[0:64, p, jt * 128:(jt + 1) * 128]
            ktb = KT[64:128, p, jt * 128:(jt + 1) * 128]
            for ic in range(2):
                qs = slice(ic * 512, (ic + 1) * 512)
                nc.tensor.matmul(Sa[:, qs], kta, QT[0:64, p, qs],
                                 start=True, stop=True, tile_position=(0, 0))
                nc.tensor.matmul(Sb[:, qs], ktb, QT[64:128, p, qs],
                                 start=True, stop=True, tile_position=(64, 0))
            Ea = cpool.tile([128, 1024], BF, tag="Ea")
            Eb = cpool.tile([128, 1024], BF, tag="Eb")
            nc.scalar.activation(out=Ea[:], in_=Sa[:], func=AF.Exp,
                                 bias=bias_sb[:, jt:jt + 1], scale=1.0)
            nc.scalar.activation(out=Eb[:], in_=Sb[:], func=AF.Exp,
                                 bias=bias_sb[:, jt:jt + 1], scale=1.0)
            va = Vaug[:, jt, ha, :]
            vb = Vaug[:, jt, hb, :]
            for ic in range(2):
                qs = slice(ic * 512, (ic + 1) * 512)
                nc.tensor.matmul(Oa[:, qs], va, Ea[:, qs],
                                 start=(jt == 0), stop=(jt == Jt - 1),
                                 skip_group_check=True)
                nc.tensor.matmul(Ob[:, qs], vb, Eb[:, qs],
                                 start=(jt == 0), stop=(jt == Jt - 1),
                                 skip_group_check=True)
        # denominators -> reciprocal (lane 64) -> partition-broadcast via DRAM
        # bounce (DMA has no PSUM route and compute engines are lane-locked)
        for (hh, O_ps, part) in ((ha, Oa, 0), (hb, Ob, 1)):
            den = dpool.tile([65, 1024], F32, tag="den")
            nc.vector.reciprocal(out=den[64:65, :], in_=O_ps[64:65, :])
            nc.sync.dma_start(out=scr[hh][:], in_=den[64:65, :])
            rb = dpool.tile([64, 1024], F32, tag="rb")
            src = scr[hh][:]
            bcast = bass.AP(tensor=src.tensor, offset=src.offset,
                            ap=[[0, 64], [1, NQ]])
            nc.sync.dma_start(out=rb[:], in_=bcast)
            if part == 0:
                nc.vector.tensor_mul(out=OT[0:64, p, :], in0=O_ps[0:64, :], in1=rb[:])
            else:
                stg = dpool.tile([64, 1024], BF, tag="stg")
                nc.vector.tensor_mul(out=stg[:], in0=O_ps[0:64, :], in1=rb[:])
                nc.sync.dma_start(out=OT[64:128, p, :], in_=stg[:])

    cctx.close()  # free attention PSUM/SBUF pools

    # ---- phase D: output projection ----
    psD = ctx.enter_context(tc.tile_pool(name="psD", bufs=3, space="PSUM"))
    opool = ctx.enter_context(tc.tile_pool(name="opool", bufs=3))
    for it in range(NQt):
        po = psD.tile([128, 1024], F32, tag="po")
        for ft in range(8):
            lhs = OT[:, ft, it * 128:(it + 1) * 128]
            nc.tensor.matmul(po[:, 0:512], lhs, wo_sb[:, ft, 0:512],
                             start=(ft == 0), stop=(ft == 7))
            nc.tensor.matmul(po[:, 512:1024], lhs, wo_sb[:, ft, 512:1024],
                             start=(ft == 0), stop=(ft == 7), skip_group_check=True)
        ost = opool.tile([128, 1024], F32, tag="ost")
        nc.scalar.activation(out=ost[:], in_=po[:], func=AF.Copy)
        nc.sync.dma_start(out=out_d[it * 128:(it + 1) * 128, :], in_=ost[:])


def prepare_inputs(x, attention_mask, norm_w, norm_b, qn_w, qn_b, kn_w, kn_b,
                   Wq, bq, Wk, bk, Wv, bv, Wo):
    """Host-side sharding/folding. Returns (J, in_maps for cores 0..7)."""
    x = np.asarray(x, np.float32)
    mask = np.asarray(attention_mask)
    for nm, a in (("norm_b", norm_b), ("bq", bq), ("bk", bk), ("bv", bv),
                  ("qn_b", qn_b), ("kn_b", kn_b)):
        assert np.abs(np.asarray(a)).max() == 0.0, f"{nm} != 0 unsupported"
    for nm, a in (("qn_w", qn_w), ("kn_w", kn_w)):
        assert np.abs(np.asarray(a) - 1.0).max() == 0.0, f"{nm} != 1 unsupported"
    norm_w = np.asarray(norm_w, np.float32)

    counts = mask.sum(1)
    J = int(max(128, np.ceil(counts.max() / 128) * 128))
    Jt = J // 128

    wqT = np.ascontiguousarray((np.asarray(Wq) * norm_w[None, :]).T).astype(BF16)
    wkT = np.ascontiguousarray((np.asarray(Wk) * norm_w[None, :]).T).astype(BF16)
    wvT = np.ascontiguousarray((np.asarray(Wv) * norm_w[None, :]).T).astype(BF16)
    woT = np.ascontiguousarray(np.asarray(Wo).T).astype(BF16)

    in_maps = []
    for c in range(8):
        b, half = c // 2, c % 2
        idx = np.flatnonzero(mask[b])
        pad = J - len(idx)
        idxp = np.concatenate([idx, np.zeros(pad, np.int64)])
        bias = np.concatenate([np.zeros(len(idx), np.float32),
                               np.full(pad, MASK_BIAS, np.float32)])
        biasT = np.ascontiguousarray(bias.reshape(Jt, 128).T)
        in_maps.append({
            "xq": np.ascontiguousarray(x[b, half * NQ:(half + 1) * NQ]),
            "xkv": np.ascontiguousarray(x[b][idxp]),
            "wqT": wqT, "wkT": wkT, "wvT": wvT, "woT": woT,
            "biasT": biasT,
        })
    return J, in_maps


_CACHE = {}


def kernel(**inputs) -> np.ndarray:
    J, in_maps = prepare_inputs(**inputs)
    key = (J, 1)
    if key not in _CACHE:
        _CACHE[key] = build_kernel(J, reps=1)
    nc = _CACHE[key]
    res = run_bass_kernel_spmd(nc, in_maps, list(range(8)))
    out = np.empty((B, N, D), np.float32)
    for c in range(8):
        b, half = c // 2, c % 2
        out[b, half * NQ:(half + 1) * NQ] = res.results[c]["out"]
    return out
